# revision 25
# baseline (speedup 1.0000x reference)
"""Trainium2 Bass kernel: Ernie4.5 VisionAttention (varlen attention, 4x512
segments, 16 heads x 80 dim, embed 1280).

Sharding: 8 cores = 2 segment-groups (2x512 tokens each) x 4 head-groups
(4 heads each). Tensor-parallel over heads (qkv column-shard, proj row-shard),
data-parallel over segment pairs. No collectives: per-core proj partials are
summed on the host.

v4: heads interleaved in the packed qk projection [q0 k0 q1 k1 ...] so head
j's rotated q/k depends only on packed tiles ~j (attention overlaps the
projection). x/wqk/wv are concatenated host-side into one "stream" tensor
loaded by five 2-chunk DMAs (dma dispatch costs ~0.6us of issuing-engine
time, and fine chunks keep the PE fed). The softmax denominator rides the
ctx matmul as a ones-column at stationary col 96 (partition 96 of the ctx
PSUM bank); 1/den is broadcast across partitions with a K=1 PE matmul
(tile_position=(96,0)). ctx rows are repacked via SBUF DMAs into three
K=128 tiles per segment so the output projection runs 3 full-K matmuls per
embed chunk. Output fp16 via 2 batched DMAs per segment.

Compute dtype: bf16 operands, fp32 PSUM accumulation.
"""

import sys

if "/opt/trn_rl_repo" not in sys.path:
    sys.path.insert(0, "/opt/trn_rl_repo")

import numpy as np
import ml_dtypes

BF = ml_dtypes.bfloat16

EMBED = 1280
HEADS = 16
HD = 80          # head dim
RH = 40          # rotary half
SEQ = 2048
SEGLEN = 512
N_CORES = 8
HPC = 4          # heads per core
TOK = 1024       # tokens per core (2 segments)
NSEG = 2
NUNITS = 2 * HPC # unit 2j = q of head j, unit 2j+1 = k of head j
VW = 97          # v block width per head (80 v + 16 pad + 1 ones col)
VTOT = HPC * VW  # 388
SW = TOK + NUNITS * HD + VTOT  # stream row: xt | wqk | wv = 1024+640+388
SCALE = HD ** -0.5
KCH = EMBED // 128  # 10

_CACHE = {}

# unpack pieces: packed row 80u+d lives in tile t=(80u+d)//128; piece list
# per packed tile t: (unit, unit_row_offset, tile_row_offset, length)
UNPACK_PIECES = {t: [] for t in range(5)}
for _u in range(NUNITS):
    _a = HD * _u
    while _a < HD * (_u + 1):
        _t = _a // 128
        _b = min(HD * (_u + 1), 128 * (_t + 1))
        UNPACK_PIECES[_t].append((_u, _a - HD * _u, _a - 128 * _t, _b - _a))
        _a = _b

# pi-swap source blocks per packed tile t (rows shift by +-40 inside each
# 80-row unit => sources span tiles t-1..t+1)
PI_BLOCKS = {0: [0, 1], 1: [0, 1, 2], 2: [1, 2, 3], 3: [2, 3, 4], 4: [3, 4]}

# rotary swap-half pieces (unused in the matmul-swap variant): where
# swap(d) = d+40 (d<40) / d-40 (d>=40); sign folded into the sin multiplier.
# (unit, dst_row, src_tile, src_row, len, emit_tile) -- emit_tile is the
# packed tile index at which both the a-write sources and the b source tile
# are available (b must land after the a-copy of the same rows).
B_PIECES = []
for _u in range(NUNITS):
    for (_d0, _g0) in ((0, HD * _u + RH), (RH, HD * _u)):
        _a = _g0
        while _a < _g0 + RH:
            _tb = _a // 128
            _b = min(_g0 + RH, 128 * (_tb + 1))
            _ln = _b - _a
            _dd = _d0 + (_a - _g0)
            _ta = (HD * _u + _dd + _ln - 1) // 128
            B_PIECES.append((_u, _dd, _tb, _a - 128 * _tb, _ln,
                             max(_ta, _tb)))
            _a = _b

# ctx repack: head j rows 80j..80j+80 -> tile 80j//128 etc.
# (unit-row-offset, dst tile, dst row, length) pieces per head
CTXP_PIECES = {j: [] for j in range(HPC)}
for _j in range(HPC):
    _a = HD * _j
    while _a < HD * (_j + 1):
        _t = _a // 128
        _b = min(HD * (_j + 1), 128 * (_t + 1))
        CTXP_PIECES[_j].append((_a - HD * _j, _t, _a - 128 * _t, _b - _a))
        _a = _b


def _build_program():
    import concourse.tile as tile
    from concourse import bacc, mybir

    f32 = mybir.dt.float32
    f16 = mybir.dt.float16
    bf16 = mybir.dt.bfloat16
    AF = mybir.ActivationFunctionType
    ALU = mybir.AluOpType

    nc = bacc.Bacc("TRN2", target_bir_lowering=False, debug=False,
                   num_devices=N_CORES)

    stream_d = nc.dram_tensor("stream", [EMBED, SW], bf16,
                              kind="ExternalInput").ap()
    vpat_d = nc.dram_tensor("vpat", [128, VTOT], bf16,
                            kind="ExternalInput").ap()
    wp_d = nc.dram_tensor("wp", [128, 3 * EMBED], bf16,
                          kind="ExternalInput").ap()
    bias_d = nc.dram_tensor("biasqk", [128, 5], f32, kind="ExternalInput").ap()
    cos_d = nc.dram_tensor("cosm", [NUNITS * HD, TOK], bf16,
                           kind="ExternalInput").ap()
    sin_d = nc.dram_tensor("sinm", [NUNITS * HD, TOK], bf16,
                           kind="ExternalInput").ap()
    pit_d = nc.dram_tensor("pit", [NUNITS * HD, NUNITS * HD], bf16,
                           kind="ExternalInput").ap()
    out_d = nc.dram_tensor("outT", [EMBED, TOK], f16, kind="ExternalOutput").ap()

    def load_grouped(dst_tile, src_ap, col_w, groups):
        for e0, e1 in groups:
            src = src_ap[128 * e0:128 * e1, :].rearrange(
                "(e p) t -> p e t", p=128)
            dst = dst_tile[:, col_w * e0:col_w * e1].rearrange(
                "p (e t) -> p e t", t=col_w)
            nc.sync.dma_start(dst, src)

    with tile.TileContext(nc) as tc:
        with tc.tile_pool(name="persist", bufs=1) as P:
            # ---- batched persistent loads (14 dma dispatches), in
            # consumption order ----
            str_sb = P.tile([128, KCH * SW], bf16, name="str_sb", tag="str")
            cos_sb = P.tile([128, 5 * TOK], bf16, name="cos_sb", tag="cos")
            sin_sb = P.tile([128, 5 * TOK], bf16, name="sin_sb", tag="sin")
            pit_sb = P.tile([128, 5 * NUNITS * HD], bf16, name="pit_sb",
                            tag="pit")
            bias_sb = P.tile([128, 5], f32, name="biasqk_sb", tag="biasqk")
            vpat_sb = P.tile([128, VTOT], bf16, name="vpat_sb", tag="vpat")
            load_grouped(str_sb, stream_d, SW,
                         [(0, 1), (1, 2), (2, 4), (4, 6), (6, 8), (8, 9), (9, 10)])
            load_grouped(pit_sb, pit_d, NUNITS * HD, [(0, 2)])
            load_grouped(cos_sb, cos_d, TOK, [(0, 2)])
            load_grouped(sin_sb, sin_d, TOK, [(0, 2)])
            nc.sync.dma_start(bias_sb[:], bias_d[:])
            nc.sync.dma_start(vpat_sb[:], vpat_d[:])
            load_grouped(pit_sb, pit_d, NUNITS * HD, [(2, 5)])
            load_grouped(cos_sb, cos_d, TOK, [(2, 5)])
            load_grouped(sin_sb, sin_d, TOK, [(2, 5)])
            wp_sb = P.tile([128, 3 * EMBED], bf16, name="wp_sb", tag="wp")
            nc.sync.dma_start(wp_sb[:], wp_d[:])

            def xt(e):
                return str_sb[:, SW * e:SW * e + TOK]

            def wqk(e):
                o = SW * e + TOK
                return str_sb[:, o:o + NUNITS * HD]

            def wv(e):
                o = SW * e + TOK + NUNITS * HD
                return str_sb[:, o:o + VTOT]

            # persistent intermediates
            qkp_sb = [[None] * NSEG for _ in range(5)]
            qrot = [P.tile([HD, TOK], bf16, name=f"qrot{u}", tag=f"qrot{u}")
                    for u in range(NUNITS)]
            v_sb = [P.tile([128, VTOT], bf16, name=f"vsb{m}", tag=f"vsb{m}")
                    for m in range(TOK // 128)]
            # repacked ctx: 3 tiles of 128 rows per segment (last 64 zero)
            ctxp = [[P.tile([128, SEGLEN], bf16, name=f"ctxp{c}_{s}",
                            tag=f"ctxp{c}_{s}") for c in range(3)]
                    for s in range(NSEG)]
            o_sb = [P.tile([128, KCH * SEGLEN], f16, name=f"osb{s}",
                           tag=f"osb{s}") for s in range(NSEG)]

            # PSUM budget (8 banks): qk-proj, pi-swap and out-proj share a
            # 3-slot pool; v + 1/den broadcast 1; scores 2; ctx 2.
            with tc.tile_pool(name="ps_a", bufs=3, space="PSUM") as PSA, \
                 tc.tile_pool(name="ps_v", bufs=1, space="PSUM") as PSV, \
                 tc.tile_pool(name="ps_st", bufs=2, space="PSUM") as PST, \
                 tc.tile_pool(name="ps_ctx", bufs=2, space="PSUM") as PSC, \
                 tc.tile_pool(name="work", bufs=3) as W, \
                 tc.tile_pool(name="workd", bufs=6) as WD:

                ones80 = P.tile([128, HD], f32, name="ones80", tag="ones80")
                nc.vector.memset(ones80[:], 1.0)
                # zero the 64 tail rows of the last ctx-repack tiles once
                for s in range(NSEG):
                    nc.vector.memset(ctxp[s][2][64:128, :], 0.0)

                est = {}    # (s, j) -> list of 4 exp'd score tiles

                def qkproj(t, s):
                    sc = slice(SEGLEN * s, SEGLEN * (s + 1))
                    if s == 0 and t >= 3:
                        qk_ps = PST.tile([128, SEGLEN], f32,
                                         name=f"qkps{t}_{s}", tag="stps")
                    else:
                        qk_ps = PSA.tile([128, SEGLEN], f32,
                                         name=f"qkps{t}_{s}", tag="mm512")
                    for e in range(KCH):
                        nc.tensor.matmul(qk_ps[:],
                                         wqk(e)[:, 128 * t:128 * (t + 1)],
                                         xt(e)[:, sc],
                                         start=(e == 0), stop=(e == KCH - 1))
                    q_sb = W.tile([128, SEGLEN], bf16, name=f"qsb{t}_{s}",
                                  tag="qsb", bufs=10)
                    nc.scalar.activation(q_sb[:], qk_ps[:], AF.Identity,
                                         bias=bias_sb[:, t:t + 1])
                    qkp_sb[t][s] = q_sb

                def vchunk(m):
                    mc = slice(128 * m, 128 * (m + 1))
                    v_ps = PSV.tile([128, VTOT], f32, name=f"vps{m}", tag="vps")
                    for e in range(KCH):
                        nc.tensor.matmul(v_ps[:], xt(e)[:, mc], wv(e),
                                         start=(e == 0), stop=(e == KCH - 1))
                    # v_sb = v_ps + (v-bias | ones) row pattern
                    nc.vector.scalar_tensor_tensor(v_sb[m][:], v_ps[:], 1.0,
                                                   vpat_sb[:], ALU.mult,
                                                   ALU.add)

                def rotary(tr, s):
                    sc = slice(SEGLEN * s, SEGLEN * (s + 1))
                    qsw_ps = PSA.tile([128, SEGLEN], f32, name=f"qsw{tr}_{s}",
                                      tag="mm512")
                    srcs = PI_BLOCKS[tr]
                    for i, tp in enumerate(srcs):
                        nc.tensor.matmul(qsw_ps[:],
                                         pit_sb[:, NUNITS * HD * tp + 128 * tr:
                                                NUNITS * HD * tp + 128 * (tr + 1)],
                                         qkp_sb[tp][s][:],
                                         start=(i == 0),
                                         stop=(i == len(srcs) - 1))
                    t1 = W.tile([128, SEGLEN], bf16, name=f"t1_{tr}_{s}",
                                tag="t1", bufs=4)
                    nc.vector.tensor_tensor(t1[:], qkp_sb[tr][s][:],
                                            cos_sb[:, TOK * tr + sc.start:
                                                   TOK * tr + sc.stop],
                                            ALU.mult)
                    t2 = W.tile([128, SEGLEN], bf16, name=f"t2_{tr}_{s}",
                                tag="t2", bufs=4)
                    nc.vector.tensor_tensor(t2[:], qsw_ps[:],
                                            sin_sb[:, TOK * tr + sc.start:
                                                   TOK * tr + sc.stop],
                                            ALU.mult)
                    rp = W.tile([128, SEGLEN], bf16, name=f"rotp{tr}_{s}",
                                tag="rotp", bufs=6)
                    nc.vector.tensor_tensor(rp[:], t1[:], t2[:], ALU.add)
                    for (u, po, toff, ln) in UNPACK_PIECES[tr]:
                        nc.sync.dma_start(qrot[u][po:po + ln, sc],
                                          rp[toff:toff + ln, :])

                def scores(s, j):
                    sc = slice(SEGLEN * s, SEGLEN * (s + 1))
                    lst = []
                    for tkc in range(SEGLEN // 128):
                        kc = slice(SEGLEN * s + 128 * tkc,
                                   SEGLEN * s + 128 * (tkc + 1))
                        st_ps = PST.tile([128, SEGLEN], f32,
                                         name=f"st{j}_{s}_{tkc}", tag="stps")
                        nc.tensor.matmul(st_ps[:], qrot[2 * j + 1][:, kc],
                                         qrot[2 * j][:, sc],
                                         start=True, stop=True)
                        e_sb = WD.tile([128, SEGLEN], bf16,
                                       name=f"est{j}_{s}_{tkc}", tag="est",
                                       bufs=14)
                        nc.scalar.activation(e_sb[:], st_ps[:], AF.Exp)
                        lst.append(e_sb)
                    est[(s, j)] = lst

                def ctx(s, j):
                    lst = est.pop((s, j))
                    ctx_ps = PSC.tile([128, SEGLEN], f32, name=f"ctxps{j}_{s}",
                                      tag="ctxps")
                    for tkc in range(SEGLEN // 128):
                        nc.tensor.matmul(ctx_ps[0:VW, :],
                                         v_sb[4 * s + tkc][:, VW * j:VW * (j + 1)],
                                         lst[tkc][:],
                                         start=(tkc == 0), stop=(tkc == 3))
                    # den accumulated into partition 96 (ones column of the
                    # v block); 1/den broadcast to 0..79 via K=1 matmul
                    rec = WD.tile([128, SEGLEN], f32, name=f"rec{j}_{s}",
                                  tag="rec", bufs=2)
                    nc.vector.reciprocal_approx_fast(rec[:, :], ctx_ps[:, :])
                    bc_ps = PSV.tile([HD, SEGLEN], f32, name=f"bc{j}_{s}",
                                     tag="vps")
                    nc.tensor.matmul(bc_ps[:], ones80[96:97, :], rec[96:97, :],
                                     start=True, stop=True,
                                     tile_position=(96, 0))
                    ctx_sb = WD.tile([HD, SEGLEN], f32, name=f"ctxsb{j}_{s}",
                                     tag="ctxsb", bufs=2)
                    if j % 2 == 0:
                        nc.scalar.activation(ctx_sb[:], ctx_ps[0:HD, :],
                                             AF.Identity)
                    else:
                        nc.vector.tensor_copy(ctx_sb[:], ctx_ps[0:HD, :])
                    ctxn = WD.tile([HD, SEGLEN], bf16, name=f"ctxn{j}_{s}",
                                   tag="ctxn", bufs=3)
                    nc.vector.tensor_tensor(ctxn[:], ctx_sb[:], bc_ps[:],
                                            ALU.mult)
                    # repack into the 3 K=128 out-proj tiles
                    for (po, c, toff, ln) in CTXP_PIECES[j]:
                        eng = nc.scalar if j == 3 else nc.sync
                        eng.dma_start(ctxp[s][c][toff:toff + ln, :],
                                      ctxn[po:po + ln, :])

                def oproj(e, s):
                    o_ps = PSA.tile([128, SEGLEN], f32, name=f"ops{e}_{s}",
                                    tag="mm512")
                    for c in range(3):
                        nc.tensor.matmul(o_ps[:],
                                         wp_sb[:, EMBED * c + 128 * e:
                                               EMBED * c + 128 * (e + 1)],
                                         ctxp[s][c][:],
                                         start=(c == 0), stop=(c == 2))
                    oc = o_sb[s][:, SEGLEN * e:SEGLEN * (e + 1)]
                    if e % 2 == 0:
                        nc.vector.tensor_copy(oc, o_ps[:])
                    else:
                        nc.scalar.activation(oc, o_ps[:], AF.Identity)

                def ostore(s, e0, e1):
                    src = o_sb[s][:, SEGLEN * e0:SEGLEN * e1].rearrange(
                        "p (e t) -> p e t", t=SEGLEN)
                    dst = out_d[128 * e0:128 * e1,
                                SEGLEN * s:SEGLEN * (s + 1)].rearrange(
                        "(e p) t -> p e t", p=128)
                    nc.sync.dma_start(dst, src)

                # ---- segment-serialized phases: all of seg-0's
                # projection + rotary + attention first (attention starts
                # after ~3 packed tiles), then seg-1's pass doubles as PE
                # filler for seg-0's out-proj and vice versa ----
                VCHS = {0: {0: [0, 1], 1: [2, 3]}, 1: {0: [4, 5], 1: [6, 7]}}

                def bpass(s):
                    for t in range(5):
                        qkproj(t, s)
                        for m in VCHS[s].get(t, []):
                            vchunk(m)
                        ready = [t - 1] if t < 4 else [3, 4]
                        for tr in ready:
                            if tr >= 0:
                                rotary(tr, s)
                        if t >= 2:
                            scores(s, t - 2)
                            ctx(s, t - 2)
                            if s == 1:
                                oproj(2 * (t - 2), 0)
                                oproj(2 * (t - 2) + 1, 0)
                    scores(s, 3)
                    ctx(s, 3)

                bpass(0)
                bpass(1)
                ostore(0, 0, 5)
                oproj(6, 0)
                oproj(7, 0)
                oproj(8, 0)
                oproj(9, 0)
                ostore(0, 5, KCH)
                for e in range(KCH):
                    oproj(e, 1)
                    if e == 4:
                        ostore(1, 0, 5)
                    if e == 8:
                        ostore(1, 5, 8)
                ostore(1, 8, KCH)

    nc.compile()
    return nc


def _prep_inputs(x, rotary_pos_emb, qkv_w, qkv_b):
    """Build per-core input shards (host-side layout/constant prep)."""
    x2 = np.asarray(x, np.float32).reshape(SEQ, EMBED)
    rope = np.asarray(rotary_pos_emb, np.float32)
    qkv_w = np.asarray(qkv_w, np.float32)
    qkv_b = np.asarray(qkv_b, np.float32)

    # packed rotary multipliers: packed row p = 80u + d -> r = d % 40
    r_idx = np.tile(np.arange(HD) % RH, NUNITS)      # [640]
    cos_full = np.cos(rope)[:, r_idx].T.astype(BF)   # [640, 2048]
    sin_full = np.sin(rope)[:, r_idx].T.astype(BF)

    # packed swap permutation (sign folded), block-diagonal per 80-row unit
    D = NUNITS * HD
    Pi = np.zeros((D, D), np.float32)
    for u in range(NUNITS):
        o = HD * u
        for i in range(RH):
            Pi[o + i, o + i + RH] = -1.0
            Pi[o + i + RH, o + i] = 1.0
    pit = np.ascontiguousarray(Pi.T).astype(BF)

    in_maps = []
    for c in range(N_CORES):
        sg, hg = divmod(c, HPC)
        toks = slice(TOK * sg, TOK * (sg + 1))
        heads = [HPC * hg + j for j in range(HPC)]

        xa = x2[toks].T                                   # [1280, 1024]

        # interleaved packing: unit 2j = q of head j, unit 2j+1 = k
        wqk = np.empty((EMBED, NUNITS * HD), np.float32)
        bias_flat = np.empty(NUNITS * HD, np.float32)
        for j, h in enumerate(heads):
            oq, ok = HD * 2 * j, HD * (2 * j + 1)
            wqk[:, oq:oq + HD] = qkv_w[HD * h:HD * (h + 1), :].T * SCALE
            bias_flat[oq:oq + HD] = qkv_b[HD * h:HD * (h + 1)] * SCALE
            ko = EMBED + HD * h
            wqk[:, ok:ok + HD] = qkv_w[ko:ko + HD, :].T
            bias_flat[ok:ok + HD] = qkv_b[ko:ko + HD]
        bias = np.ascontiguousarray(bias_flat.reshape(5, 128).T)

        # v weights: 97-wide blocks per head (80 v | 16 zero | 1 zero);
        # eviction adds vpat = (v-bias | 0 | 1.0) so the ctx matmul
        # accumulates the softmax denominator into partition 96
        wv = np.zeros((EMBED, VTOT), np.float32)
        vpat_row = np.zeros(VTOT, np.float32)
        for j, h in enumerate(heads):
            vo = 2 * EMBED + HD * h
            wv[:, VW * j:VW * j + HD] = qkv_w[vo:vo + HD, :].T
            vpat_row[VW * j:VW * j + HD] = qkv_b[vo:vo + HD]
            vpat_row[VW * j + 96] = 1.0
        vpat = np.ascontiguousarray(np.broadcast_to(vpat_row, (128, VTOT)))

        stream = np.concatenate([xa, wqk, wv], axis=1)    # [1280, SW]

        # wp packed for K=128 repacked ctx: rows = stacked head-dims
        # (320 real + 64 zero), 3 chunks of 128 side by side
        wp_cat = np.zeros((384, EMBED), np.float32)
        for j, h in enumerate(heads):
            wp_cat[HD * j:HD * (j + 1), :] = _PROJ_W[:, HD * h:HD * (h + 1)].T
        wp = np.zeros((128, 3 * EMBED), np.float32)
        for c_ in range(3):
            wp[:, EMBED * c_:EMBED * (c_ + 1)] = wp_cat[128 * c_:128 * (c_ + 1)]

        in_maps.append({
            "stream": np.ascontiguousarray(stream).astype(BF),
            "vpat": vpat.astype(BF),
            "wp": np.ascontiguousarray(wp).astype(BF),
            "biasqk": bias,
            "cosm": np.ascontiguousarray(cos_full[:, toks]),
            "sinm": np.ascontiguousarray(sin_full[:, toks]),
            "pit": pit,
        })
    return in_maps


_PROJ_W = None


def run_on_device(inputs, trace=False, trace_cores=None):
    """Shard, run on 8 NeuronCores, gather. Returns (output, BassKernelResults)."""
    global _PROJ_W
    from concourse import bass_utils

    x = np.asarray(inputs["x"], np.float32)
    cu = np.asarray(inputs["cu_seqlens"]).tolist()
    assert cu == [0, 512, 1024, 1536, 2048], (
        f"kernel compiled for 4x512 segments, got cu_seqlens={cu}")
    assert x.shape == (SEQ, 1, EMBED)

    _PROJ_W = np.asarray(inputs["proj_w"], np.float32)
    in_maps = _prep_inputs(x, inputs["rotary_pos_emb"],
                           inputs["qkv_w"], inputs["qkv_b"])

    if "nc" not in _CACHE:
        _CACHE["nc"] = _build_program()
    nc = _CACHE["nc"]

    kw = {}
    if trace:
        kw = dict(trace=True, trace_cores=trace_cores or [0])
    res = bass_utils.run_bass_kernel_spmd(nc, in_maps,
                                          core_ids=list(range(N_CORES)), **kw)

    proj_b = np.asarray(inputs["proj_b"], np.float32)
    out = np.empty((SEQ, EMBED), np.float32)
    for sg in range(2):
        acc = res.results[HPC * sg + 0]["outT"].astype(np.float32)
        for hg in range(1, HPC):
            acc = acc + res.results[HPC * sg + hg]["outT"].astype(np.float32)
        out[TOK * sg:TOK * (sg + 1)] = acc.T
    out += proj_b
    return out.reshape(SEQ, 1, EMBED), res


def kernel(**inputs):
    out, _ = run_on_device(inputs, trace=False)
    return out


# revision 26
# speedup vs baseline: 1.1017x; 1.1017x over previous
"""Trainium2 Bass kernel: Ernie4.5 VisionAttention (varlen attention, 4x512
segments, 16 heads x 80 dim, embed 1280).

Sharding: 8 cores = 2 segment-groups (2x512 tokens each) x 4 head-groups
(4 heads each). Tensor-parallel over heads (qkv column-shard, proj row-shard),
data-parallel over segment pairs. No collectives: per-core proj partials are
summed on the host.

v4: heads interleaved in the packed qk projection [q0 k0 q1 k1 ...] so head
j's rotated q/k depends only on packed tiles ~j (attention overlaps the
projection). x/wqk/wv are concatenated host-side into one "stream" tensor
loaded by five 2-chunk DMAs (dma dispatch costs ~0.6us of issuing-engine
time, and fine chunks keep the PE fed). The softmax denominator rides the
ctx matmul as a ones-column at stationary col 96 (partition 96 of the ctx
PSUM bank); 1/den is broadcast across partitions with a K=1 PE matmul
(tile_position=(96,0)). ctx rows are repacked via SBUF DMAs into three
K=128 tiles per segment so the output projection runs 3 full-K matmuls per
embed chunk. Output fp16 via 2 batched DMAs per segment.

Compute dtype: bf16 operands, fp32 PSUM accumulation.
"""

import sys

if "/opt/trn_rl_repo" not in sys.path:
    sys.path.insert(0, "/opt/trn_rl_repo")

import numpy as np
import ml_dtypes

BF = ml_dtypes.bfloat16

EMBED = 1280
HEADS = 16
HD = 80          # head dim
RH = 40          # rotary half
SEQ = 2048
SEGLEN = 512
N_CORES = 8
HPC = 4          # heads per core
TOK = 1024       # tokens per core (2 segments)
NSEG = 2
NUNITS = 2 * HPC # unit 2j = q of head j, unit 2j+1 = k of head j
VW = 97          # v block width per head (80 v + 16 pad + 1 ones col)
VTOT = HPC * VW  # 388
SW = TOK + NUNITS * HD + VTOT  # stream row: xt | wqk | wv = 1024+640+388
SCALE = HD ** -0.5
KCH = EMBED // 128  # 10

_CACHE = {}

# unpack pieces: packed row 80u+d lives in tile t=(80u+d)//128; piece list
# per packed tile t: (unit, unit_row_offset, tile_row_offset, length)
UNPACK_PIECES = {t: [] for t in range(5)}
for _u in range(NUNITS):
    _a = HD * _u
    while _a < HD * (_u + 1):
        _t = _a // 128
        _b = min(HD * (_u + 1), 128 * (_t + 1))
        UNPACK_PIECES[_t].append((_u, _a - HD * _u, _a - 128 * _t, _b - _a))
        _a = _b

# pi-swap source blocks per packed tile t (rows shift by +-40 inside each
# 80-row unit => sources span tiles t-1..t+1)
PI_BLOCKS = {0: [0, 1], 1: [0, 1, 2], 2: [1, 2, 3], 3: [2, 3, 4], 4: [3, 4]}

# rotary swap-half pieces (unused in the matmul-swap variant): where
# swap(d) = d+40 (d<40) / d-40 (d>=40); sign folded into the sin multiplier.
# (unit, dst_row, src_tile, src_row, len, emit_tile) -- emit_tile is the
# packed tile index at which both the a-write sources and the b source tile
# are available (b must land after the a-copy of the same rows).
B_PIECES = []
for _u in range(NUNITS):
    for (_d0, _g0) in ((0, HD * _u + RH), (RH, HD * _u)):
        _a = _g0
        while _a < _g0 + RH:
            _tb = _a // 128
            _b = min(_g0 + RH, 128 * (_tb + 1))
            _ln = _b - _a
            _dd = _d0 + (_a - _g0)
            _ta = (HD * _u + _dd + _ln - 1) // 128
            B_PIECES.append((_u, _dd, _tb, _a - 128 * _tb, _ln,
                             max(_ta, _tb)))
            _a = _b

# ctx repack: head j rows 80j..80j+80 -> tile 80j//128 etc.
# (unit-row-offset, dst tile, dst row, length) pieces per head
CTXP_PIECES = {j: [] for j in range(HPC)}
for _j in range(HPC):
    _a = HD * _j
    while _a < HD * (_j + 1):
        _t = _a // 128
        _b = min(HD * (_j + 1), 128 * (_t + 1))
        CTXP_PIECES[_j].append((_a - HD * _j, _t, _a - 128 * _t, _b - _a))
        _a = _b


def _build_program():
    import concourse.tile as tile
    from concourse import bacc, mybir

    f32 = mybir.dt.float32
    f16 = mybir.dt.float16
    bf16 = mybir.dt.bfloat16
    AF = mybir.ActivationFunctionType
    ALU = mybir.AluOpType

    nc = bacc.Bacc("TRN2", target_bir_lowering=False, debug=False,
                   num_devices=N_CORES)

    stream_d = nc.dram_tensor("stream", [EMBED, SW], bf16,
                              kind="ExternalInput").ap()
    vpat_d = nc.dram_tensor("vpat", [128, VTOT], bf16,
                            kind="ExternalInput").ap()
    wp_d = nc.dram_tensor("wp", [128, 3 * EMBED], bf16,
                          kind="ExternalInput").ap()
    bias_d = nc.dram_tensor("biasqk", [128, 5], f32, kind="ExternalInput").ap()
    cos_d = nc.dram_tensor("cosm", [NUNITS * HD, TOK], bf16,
                           kind="ExternalInput").ap()
    sin_d = nc.dram_tensor("sinm", [NUNITS * HD, TOK], bf16,
                           kind="ExternalInput").ap()
    pit_d = nc.dram_tensor("pit", [NUNITS * HD, NUNITS * HD], bf16,
                           kind="ExternalInput").ap()
    out_d = nc.dram_tensor("outT", [EMBED, TOK], f16, kind="ExternalOutput").ap()

    def load_grouped(dst_tile, src_ap, col_w, groups):
        for e0, e1 in groups:
            src = src_ap[128 * e0:128 * e1, :].rearrange(
                "(e p) t -> p e t", p=128)
            dst = dst_tile[:, col_w * e0:col_w * e1].rearrange(
                "p (e t) -> p e t", t=col_w)
            nc.sync.dma_start(dst, src)

    with tile.TileContext(nc) as tc:
        with tc.tile_pool(name="persist", bufs=1) as P:
            # ---- batched persistent loads (14 dma dispatches), in
            # consumption order ----
            str_sb = P.tile([128, KCH * SW], bf16, name="str_sb", tag="str")
            cos_sb = P.tile([128, 5 * TOK], bf16, name="cos_sb", tag="cos")
            sin_sb = P.tile([128, 5 * TOK], bf16, name="sin_sb", tag="sin")
            pit_sb = P.tile([128, 5 * NUNITS * HD], bf16, name="pit_sb",
                            tag="pit")
            bias_sb = P.tile([128, 5], f32, name="biasqk_sb", tag="biasqk")
            vpat_sb = P.tile([128, VTOT], bf16, name="vpat_sb", tag="vpat")
            load_grouped(str_sb, stream_d, SW,
                         [(0, 1), (1, 2), (2, 4), (4, 6), (6, 8), (8, 10)])
            load_grouped(pit_sb, pit_d, NUNITS * HD, [(0, 2)])
            load_grouped(cos_sb, cos_d, TOK, [(0, 2)])
            load_grouped(sin_sb, sin_d, TOK, [(0, 2)])
            nc.sync.dma_start(bias_sb[:], bias_d[:])
            nc.sync.dma_start(vpat_sb[:], vpat_d[:])
            load_grouped(pit_sb, pit_d, NUNITS * HD, [(2, 5)])
            load_grouped(cos_sb, cos_d, TOK, [(2, 5)])
            load_grouped(sin_sb, sin_d, TOK, [(2, 5)])
            wp_sb = P.tile([128, 3 * EMBED], bf16, name="wp_sb", tag="wp")
            nc.sync.dma_start(wp_sb[:], wp_d[:])

            def xt(e):
                return str_sb[:, SW * e:SW * e + TOK]

            def wqk(e):
                o = SW * e + TOK
                return str_sb[:, o:o + NUNITS * HD]

            def wv(e):
                o = SW * e + TOK + NUNITS * HD
                return str_sb[:, o:o + VTOT]

            # persistent intermediates
            qkp_sb = [[None] * NSEG for _ in range(5)]
            qrot = [P.tile([HD, TOK], bf16, name=f"qrot{u}", tag=f"qrot{u}")
                    for u in range(NUNITS)]
            v_sb = [P.tile([128, VTOT], bf16, name=f"vsb{m}", tag=f"vsb{m}")
                    for m in range(TOK // 128)]
            # repacked ctx: 3 tiles of 128 rows per segment (last 64 zero)
            ctxp = [[P.tile([128, SEGLEN], bf16, name=f"ctxp{c}_{s}",
                            tag=f"ctxp{c}_{s}") for c in range(3)]
                    for s in range(NSEG)]
            o_sb = [P.tile([128, KCH * SEGLEN], f16, name=f"osb{s}",
                           tag=f"osb{s}") for s in range(NSEG)]

            # PSUM budget (8 banks): qk-proj, pi-swap and out-proj share a
            # 3-slot pool; v + 1/den broadcast 1; scores 2; ctx 2.
            with tc.tile_pool(name="ps_a", bufs=3, space="PSUM") as PSA, \
                 tc.tile_pool(name="ps_v", bufs=1, space="PSUM") as PSV, \
                 tc.tile_pool(name="ps_st", bufs=2, space="PSUM") as PST, \
                 tc.tile_pool(name="ps_ctx", bufs=2, space="PSUM") as PSC, \
                 tc.tile_pool(name="work", bufs=3) as W, \
                 tc.tile_pool(name="workd", bufs=6) as WD:

                ones80 = P.tile([128, HD], f32, name="ones80", tag="ones80")
                nc.vector.memset(ones80[:], 1.0)
                # zero the 64 tail rows of the last ctx-repack tiles once
                for s in range(NSEG):
                    nc.vector.memset(ctxp[s][2][64:128, :], 0.0)

                est = {}    # (s, j) -> list of 4 exp'd score tiles

                def qkproj(t, s):
                    sc = slice(SEGLEN * s, SEGLEN * (s + 1))
                    qk_ps = PSA.tile([128, SEGLEN], f32, name=f"qkps{t}_{s}",
                                     tag="mm512")
                    for e in range(KCH):
                        nc.tensor.matmul(qk_ps[:],
                                         wqk(e)[:, 128 * t:128 * (t + 1)],
                                         xt(e)[:, sc],
                                         start=(e == 0), stop=(e == KCH - 1))
                    q_sb = W.tile([128, SEGLEN], bf16, name=f"qsb{t}_{s}",
                                  tag="qsb", bufs=10)
                    nc.scalar.activation(q_sb[:], qk_ps[:], AF.Identity,
                                         bias=bias_sb[:, t:t + 1])
                    qkp_sb[t][s] = q_sb

                def vchunk(m):
                    mc = slice(128 * m, 128 * (m + 1))
                    v_ps = PSV.tile([128, VTOT], f32, name=f"vps{m}", tag="vps")
                    for e in range(KCH):
                        nc.tensor.matmul(v_ps[:], xt(e)[:, mc], wv(e),
                                         start=(e == 0), stop=(e == KCH - 1))
                    # v_sb = v_ps + (v-bias | ones) row pattern
                    nc.vector.scalar_tensor_tensor(v_sb[m][:], v_ps[:], 1.0,
                                                   vpat_sb[:], ALU.mult,
                                                   ALU.add)

                def rotary(tr, s):
                    sc = slice(SEGLEN * s, SEGLEN * (s + 1))
                    qsw_ps = PSA.tile([128, SEGLEN], f32, name=f"qsw{tr}_{s}",
                                      tag="mm512")
                    srcs = PI_BLOCKS[tr]
                    for i, tp in enumerate(srcs):
                        nc.tensor.matmul(qsw_ps[:],
                                         pit_sb[:, NUNITS * HD * tp + 128 * tr:
                                                NUNITS * HD * tp + 128 * (tr + 1)],
                                         qkp_sb[tp][s][:],
                                         start=(i == 0),
                                         stop=(i == len(srcs) - 1))
                    t1 = W.tile([128, SEGLEN], bf16, name=f"t1_{tr}_{s}",
                                tag="t1", bufs=4)
                    nc.vector.tensor_tensor(t1[:], qkp_sb[tr][s][:],
                                            cos_sb[:, TOK * tr + sc.start:
                                                   TOK * tr + sc.stop],
                                            ALU.mult)
                    t2 = W.tile([128, SEGLEN], bf16, name=f"t2_{tr}_{s}",
                                tag="t2", bufs=4)
                    nc.vector.tensor_tensor(t2[:], qsw_ps[:],
                                            sin_sb[:, TOK * tr + sc.start:
                                                   TOK * tr + sc.stop],
                                            ALU.mult)
                    rp = W.tile([128, SEGLEN], bf16, name=f"rotp{tr}_{s}",
                                tag="rotp", bufs=6)
                    nc.vector.tensor_tensor(rp[:], t1[:], t2[:], ALU.add)
                    for (u, po, toff, ln) in UNPACK_PIECES[tr]:
                        nc.sync.dma_start(qrot[u][po:po + ln, sc],
                                          rp[toff:toff + ln, :])

                def scores(s, j):
                    sc = slice(SEGLEN * s, SEGLEN * (s + 1))
                    lst = []
                    for tkc in range(SEGLEN // 128):
                        kc = slice(SEGLEN * s + 128 * tkc,
                                   SEGLEN * s + 128 * (tkc + 1))
                        st_ps = PST.tile([128, SEGLEN], f32,
                                         name=f"st{j}_{s}_{tkc}", tag="stps")
                        nc.tensor.matmul(st_ps[:], qrot[2 * j + 1][:, kc],
                                         qrot[2 * j][:, sc],
                                         start=True, stop=True)
                        e_sb = WD.tile([128, SEGLEN], bf16,
                                       name=f"est{j}_{s}_{tkc}", tag="est",
                                       bufs=14)
                        nc.scalar.activation(e_sb[:], st_ps[:], AF.Exp)
                        lst.append(e_sb)
                    est[(s, j)] = lst

                def ctx(s, j):
                    lst = est.pop((s, j))
                    ctx_ps = PSC.tile([128, SEGLEN], f32, name=f"ctxps{j}_{s}",
                                      tag="ctxps")
                    for tkc in range(SEGLEN // 128):
                        nc.tensor.matmul(ctx_ps[0:VW, :],
                                         v_sb[4 * s + tkc][:, VW * j:VW * (j + 1)],
                                         lst[tkc][:],
                                         start=(tkc == 0), stop=(tkc == 3))
                    # den accumulated into partition 96 (ones column of the
                    # v block); 1/den broadcast to 0..79 via K=1 matmul
                    rec = WD.tile([128, SEGLEN], f32, name=f"rec{j}_{s}",
                                  tag="rec", bufs=2)
                    nc.vector.reciprocal_approx_fast(rec[:, :], ctx_ps[:, :])
                    bc_ps = PSV.tile([HD, SEGLEN], f32, name=f"bc{j}_{s}",
                                     tag="vps")
                    nc.tensor.matmul(bc_ps[:], ones80[96:97, :], rec[96:97, :],
                                     start=True, stop=True,
                                     tile_position=(96, 0))
                    ctx_sb = WD.tile([HD, SEGLEN], f32, name=f"ctxsb{j}_{s}",
                                     tag="ctxsb", bufs=2)
                    if j % 2 == 0:
                        nc.scalar.activation(ctx_sb[:], ctx_ps[0:HD, :],
                                             AF.Identity)
                    else:
                        nc.vector.tensor_copy(ctx_sb[:], ctx_ps[0:HD, :])
                    ctxn = WD.tile([HD, SEGLEN], bf16, name=f"ctxn{j}_{s}",
                                   tag="ctxn", bufs=3)
                    nc.vector.tensor_tensor(ctxn[:], ctx_sb[:], bc_ps[:],
                                            ALU.mult)
                    # repack into the 3 K=128 out-proj tiles
                    for (po, c, toff, ln) in CTXP_PIECES[j]:
                        nc.sync.dma_start(ctxp[s][c][toff:toff + ln, :],
                                          ctxn[po:po + ln, :])

                def oproj(e, s):
                    o_ps = PSA.tile([128, SEGLEN], f32, name=f"ops{e}_{s}",
                                    tag="mm512")
                    for c in range(3):
                        nc.tensor.matmul(o_ps[:],
                                         wp_sb[:, EMBED * c + 128 * e:
                                               EMBED * c + 128 * (e + 1)],
                                         ctxp[s][c][:],
                                         start=(c == 0), stop=(c == 2))
                    oc = o_sb[s][:, SEGLEN * e:SEGLEN * (e + 1)]
                    if e % 2 == 0:
                        nc.vector.tensor_copy(oc, o_ps[:])
                    else:
                        nc.scalar.activation(oc, o_ps[:], AF.Identity)

                def ostore(s, e0, e1):
                    src = o_sb[s][:, SEGLEN * e0:SEGLEN * e1].rearrange(
                        "p (e t) -> p e t", t=SEGLEN)
                    dst = out_d[128 * e0:128 * e1,
                                SEGLEN * s:SEGLEN * (s + 1)].rearrange(
                        "(e p) t -> p e t", p=128)
                    nc.sync.dma_start(dst, src)

                # ---- segment-serialized phases: all of seg-0's
                # projection + rotary + attention first (attention starts
                # after ~3 packed tiles), then seg-1's pass doubles as PE
                # filler for seg-0's out-proj and vice versa ----
                VCHS = {0: {0: [0, 1], 1: [2, 3]}, 1: {0: [4, 5], 1: [6, 7]}}

                def bpass(s):
                    for t in range(5):
                        qkproj(t, s)
                        for m in VCHS[s].get(t, []):
                            vchunk(m)
                        ready = [t - 1] if t < 4 else [3, 4]
                        for tr in ready:
                            if tr >= 0:
                                rotary(tr, s)
                        if t >= 2:
                            scores(s, t - 2)
                            ctx(s, t - 2)
                            if s == 1:
                                oproj(2 * (t - 2), 0)
                                oproj(2 * (t - 2) + 1, 0)
                    scores(s, 3)
                    ctx(s, 3)

                bpass(0)
                bpass(1)
                ostore(0, 0, 5)
                oproj(6, 0)
                oproj(7, 0)
                oproj(8, 0)
                oproj(9, 0)
                ostore(0, 5, KCH)
                for e in range(KCH):
                    oproj(e, 1)
                    if e == 4:
                        ostore(1, 0, 5)
                    if e == 8:
                        ostore(1, 5, 8)
                ostore(1, 8, KCH)

    nc.compile()
    return nc


def _prep_inputs(x, rotary_pos_emb, qkv_w, qkv_b):
    """Build per-core input shards (host-side layout/constant prep)."""
    x2 = np.asarray(x, np.float32).reshape(SEQ, EMBED)
    rope = np.asarray(rotary_pos_emb, np.float32)
    qkv_w = np.asarray(qkv_w, np.float32)
    qkv_b = np.asarray(qkv_b, np.float32)

    # packed rotary multipliers: packed row p = 80u + d -> r = d % 40
    r_idx = np.tile(np.arange(HD) % RH, NUNITS)      # [640]
    cos_full = np.cos(rope)[:, r_idx].T.astype(BF)   # [640, 2048]
    sin_full = np.sin(rope)[:, r_idx].T.astype(BF)

    # packed swap permutation (sign folded), block-diagonal per 80-row unit
    D = NUNITS * HD
    Pi = np.zeros((D, D), np.float32)
    for u in range(NUNITS):
        o = HD * u
        for i in range(RH):
            Pi[o + i, o + i + RH] = -1.0
            Pi[o + i + RH, o + i] = 1.0
    pit = np.ascontiguousarray(Pi.T).astype(BF)

    in_maps = []
    for c in range(N_CORES):
        sg, hg = divmod(c, HPC)
        toks = slice(TOK * sg, TOK * (sg + 1))
        heads = [HPC * hg + j for j in range(HPC)]

        xa = x2[toks].T                                   # [1280, 1024]

        # interleaved packing: unit 2j = q of head j, unit 2j+1 = k
        wqk = np.empty((EMBED, NUNITS * HD), np.float32)
        bias_flat = np.empty(NUNITS * HD, np.float32)
        for j, h in enumerate(heads):
            oq, ok = HD * 2 * j, HD * (2 * j + 1)
            wqk[:, oq:oq + HD] = qkv_w[HD * h:HD * (h + 1), :].T * SCALE
            bias_flat[oq:oq + HD] = qkv_b[HD * h:HD * (h + 1)] * SCALE
            ko = EMBED + HD * h
            wqk[:, ok:ok + HD] = qkv_w[ko:ko + HD, :].T
            bias_flat[ok:ok + HD] = qkv_b[ko:ko + HD]
        bias = np.ascontiguousarray(bias_flat.reshape(5, 128).T)

        # v weights: 97-wide blocks per head (80 v | 16 zero | 1 zero);
        # eviction adds vpat = (v-bias | 0 | 1.0) so the ctx matmul
        # accumulates the softmax denominator into partition 96
        wv = np.zeros((EMBED, VTOT), np.float32)
        vpat_row = np.zeros(VTOT, np.float32)
        for j, h in enumerate(heads):
            vo = 2 * EMBED + HD * h
            wv[:, VW * j:VW * j + HD] = qkv_w[vo:vo + HD, :].T
            vpat_row[VW * j:VW * j + HD] = qkv_b[vo:vo + HD]
            vpat_row[VW * j + 96] = 1.0
        vpat = np.ascontiguousarray(np.broadcast_to(vpat_row, (128, VTOT)))

        stream = np.concatenate([xa, wqk, wv], axis=1)    # [1280, SW]

        # wp packed for K=128 repacked ctx: rows = stacked head-dims
        # (320 real + 64 zero), 3 chunks of 128 side by side
        wp_cat = np.zeros((384, EMBED), np.float32)
        for j, h in enumerate(heads):
            wp_cat[HD * j:HD * (j + 1), :] = _PROJ_W[:, HD * h:HD * (h + 1)].T
        wp = np.zeros((128, 3 * EMBED), np.float32)
        for c_ in range(3):
            wp[:, EMBED * c_:EMBED * (c_ + 1)] = wp_cat[128 * c_:128 * (c_ + 1)]

        in_maps.append({
            "stream": np.ascontiguousarray(stream).astype(BF),
            "vpat": vpat.astype(BF),
            "wp": np.ascontiguousarray(wp).astype(BF),
            "biasqk": bias,
            "cosm": np.ascontiguousarray(cos_full[:, toks]),
            "sinm": np.ascontiguousarray(sin_full[:, toks]),
            "pit": pit,
        })
    return in_maps


_PROJ_W = None


def run_on_device(inputs, trace=False, trace_cores=None):
    """Shard, run on 8 NeuronCores, gather. Returns (output, BassKernelResults)."""
    global _PROJ_W
    from concourse import bass_utils

    x = np.asarray(inputs["x"], np.float32)
    cu = np.asarray(inputs["cu_seqlens"]).tolist()
    assert cu == [0, 512, 1024, 1536, 2048], (
        f"kernel compiled for 4x512 segments, got cu_seqlens={cu}")
    assert x.shape == (SEQ, 1, EMBED)

    _PROJ_W = np.asarray(inputs["proj_w"], np.float32)
    in_maps = _prep_inputs(x, inputs["rotary_pos_emb"],
                           inputs["qkv_w"], inputs["qkv_b"])

    if "nc" not in _CACHE:
        _CACHE["nc"] = _build_program()
    nc = _CACHE["nc"]

    kw = {}
    if trace:
        kw = dict(trace=True, trace_cores=trace_cores or [0])
    res = bass_utils.run_bass_kernel_spmd(nc, in_maps,
                                          core_ids=list(range(N_CORES)), **kw)

    proj_b = np.asarray(inputs["proj_b"], np.float32)
    out = np.empty((SEQ, EMBED), np.float32)
    for sg in range(2):
        acc = res.results[HPC * sg + 0]["outT"].astype(np.float32)
        for hg in range(1, HPC):
            acc = acc + res.results[HPC * sg + hg]["outT"].astype(np.float32)
        out[TOK * sg:TOK * (sg + 1)] = acc.T
    out += proj_b
    return out.reshape(SEQ, 1, EMBED), res


def kernel(**inputs):
    out, _ = run_on_device(inputs, trace=False)
    return out


# revision 27
# speedup vs baseline: 1.1427x; 1.0372x over previous
"""Trainium2 Bass kernel: Ernie4.5 VisionAttention (varlen attention, 4x512
segments, 16 heads x 80 dim, embed 1280).

Sharding: 8 cores = 2 segment-groups (2x512 tokens each) x 4 head-groups
(4 heads each). Tensor-parallel over heads (qkv column-shard, proj row-shard),
data-parallel over segment pairs. No collectives: per-core proj partials are
summed on the host.

v4: heads interleaved in the packed qk projection [q0 k0 q1 k1 ...] so head
j's rotated q/k depends only on packed tiles ~j (attention overlaps the
projection). x/wqk/wv are concatenated host-side into one "stream" tensor
loaded by five 2-chunk DMAs (dma dispatch costs ~0.6us of issuing-engine
time, and fine chunks keep the PE fed). The softmax denominator rides the
ctx matmul as a ones-column at stationary col 96 (partition 96 of the ctx
PSUM bank); 1/den is broadcast across partitions with a K=1 PE matmul
(tile_position=(96,0)). ctx rows are repacked via SBUF DMAs into three
K=128 tiles per segment so the output projection runs 3 full-K matmuls per
embed chunk. Output fp16 via 2 batched DMAs per segment.

Compute dtype: bf16 operands, fp32 PSUM accumulation.
"""

import sys

if "/opt/trn_rl_repo" not in sys.path:
    sys.path.insert(0, "/opt/trn_rl_repo")

import numpy as np
import ml_dtypes

BF = ml_dtypes.bfloat16

EMBED = 1280
HEADS = 16
HD = 80          # head dim
RH = 40          # rotary half
SEQ = 2048
SEGLEN = 512
N_CORES = 8
HPC = 4          # heads per core
TOK = 1024       # tokens per core (2 segments)
NSEG = 2
NUNITS = 2 * HPC # unit 2j = q of head j, unit 2j+1 = k of head j
VW = 97          # v block width per head in SBUF (80 v + 16 pad + 1 ones col)
VTOT = HPC * VW  # 388 (sbuf layout)
VC = HPC * HD    # 320 compact v weight width (streamed; scattered on evict)
SW = TOK + NUNITS * HD + VC  # stream row: xt | wqk | wv = 1024+640+320
SCALE = HD ** -0.5
KCH = EMBED // 128  # 10

_CACHE = {}

# unpack pieces: packed row 80u+d lives in tile t=(80u+d)//128; piece list
# per packed tile t: (unit, unit_row_offset, tile_row_offset, length)
UNPACK_PIECES = {t: [] for t in range(5)}
for _u in range(NUNITS):
    _a = HD * _u
    while _a < HD * (_u + 1):
        _t = _a // 128
        _b = min(HD * (_u + 1), 128 * (_t + 1))
        UNPACK_PIECES[_t].append((_u, _a - HD * _u, _a - 128 * _t, _b - _a))
        _a = _b

# pi-swap source blocks per packed tile t (rows shift by +-40 inside each
# 80-row unit => sources span tiles t-1..t+1)
PI_BLOCKS = {0: [0, 1], 1: [0, 1, 2], 2: [1, 2, 3], 3: [2, 3, 4], 4: [3, 4]}

# rotary swap-half pieces (unused in the matmul-swap variant): where
# swap(d) = d+40 (d<40) / d-40 (d>=40); sign folded into the sin multiplier.
# (unit, dst_row, src_tile, src_row, len, emit_tile) -- emit_tile is the
# packed tile index at which both the a-write sources and the b source tile
# are available (b must land after the a-copy of the same rows).
B_PIECES = []
for _u in range(NUNITS):
    for (_d0, _g0) in ((0, HD * _u + RH), (RH, HD * _u)):
        _a = _g0
        while _a < _g0 + RH:
            _tb = _a // 128
            _b = min(_g0 + RH, 128 * (_tb + 1))
            _ln = _b - _a
            _dd = _d0 + (_a - _g0)
            _ta = (HD * _u + _dd + _ln - 1) // 128
            B_PIECES.append((_u, _dd, _tb, _a - 128 * _tb, _ln,
                             max(_ta, _tb)))
            _a = _b

# ctx repack: head j rows 80j..80j+80 -> tile 80j//128 etc.
# (unit-row-offset, dst tile, dst row, length) pieces per head
CTXP_PIECES = {j: [] for j in range(HPC)}
for _j in range(HPC):
    _a = HD * _j
    while _a < HD * (_j + 1):
        _t = _a // 128
        _b = min(HD * (_j + 1), 128 * (_t + 1))
        CTXP_PIECES[_j].append((_a - HD * _j, _t, _a - 128 * _t, _b - _a))
        _a = _b


def _build_program():
    import concourse.tile as tile
    from concourse import bacc, mybir

    f32 = mybir.dt.float32
    f16 = mybir.dt.float16
    bf16 = mybir.dt.bfloat16
    AF = mybir.ActivationFunctionType
    ALU = mybir.AluOpType

    nc = bacc.Bacc("TRN2", target_bir_lowering=False, debug=False,
                   num_devices=N_CORES)

    stream_d = nc.dram_tensor("stream", [EMBED, SW], bf16,
                              kind="ExternalInput").ap()
    vpat_d = nc.dram_tensor("vpat", [128, VC], bf16,
                            kind="ExternalInput").ap()
    wp_d = nc.dram_tensor("wp", [128, 3 * EMBED], bf16,
                          kind="ExternalInput").ap()
    bias_d = nc.dram_tensor("biasqk", [128, 5], f32, kind="ExternalInput").ap()
    cos_d = nc.dram_tensor("cosm", [NUNITS * HD, TOK], bf16,
                           kind="ExternalInput").ap()
    sin_d = nc.dram_tensor("sinm", [NUNITS * HD, TOK], bf16,
                           kind="ExternalInput").ap()
    pit_d = nc.dram_tensor("pit", [NUNITS * HD, NUNITS * HD], bf16,
                           kind="ExternalInput").ap()
    out_d = nc.dram_tensor("outT", [EMBED, TOK], f16, kind="ExternalOutput").ap()

    def load_grouped(dst_tile, src_ap, col_w, groups):
        for e0, e1 in groups:
            src = src_ap[128 * e0:128 * e1, :].rearrange(
                "(e p) t -> p e t", p=128)
            dst = dst_tile[:, col_w * e0:col_w * e1].rearrange(
                "p (e t) -> p e t", t=col_w)
            nc.sync.dma_start(dst, src)

    with tile.TileContext(nc) as tc:
        with tc.tile_pool(name="persist", bufs=1) as P:
            # ---- batched persistent loads (14 dma dispatches), in
            # consumption order ----
            str_sb = P.tile([128, KCH * SW], bf16, name="str_sb", tag="str")
            cos_sb = P.tile([128, 5 * TOK], bf16, name="cos_sb", tag="cos")
            sin_sb = P.tile([128, 5 * TOK], bf16, name="sin_sb", tag="sin")
            pit_sb = P.tile([128, 5 * NUNITS * HD], bf16, name="pit_sb",
                            tag="pit")
            bias_sb = P.tile([128, 5], f32, name="biasqk_sb", tag="biasqk")
            vpat_sb = P.tile([128, VC], bf16, name="vpat_sb", tag="vpat")
            load_grouped(str_sb, stream_d, SW,
                         [(0, 1), (1, 2), (2, 4), (4, 6), (6, 8), (8, 10)])
            load_grouped(pit_sb, pit_d, NUNITS * HD, [(0, 2)])
            load_grouped(cos_sb, cos_d, TOK, [(0, 2)])
            load_grouped(sin_sb, sin_d, TOK, [(0, 2)])
            nc.sync.dma_start(bias_sb[:], bias_d[:])
            nc.sync.dma_start(vpat_sb[:], vpat_d[:])
            load_grouped(pit_sb, pit_d, NUNITS * HD, [(2, 5)])
            load_grouped(cos_sb, cos_d, TOK, [(2, 5)])
            load_grouped(sin_sb, sin_d, TOK, [(2, 5)])
            wp_sb = P.tile([128, 3 * EMBED], bf16, name="wp_sb", tag="wp")
            nc.sync.dma_start(wp_sb[:], wp_d[:])

            def xt(e):
                return str_sb[:, SW * e:SW * e + TOK]

            def wqk(e):
                o = SW * e + TOK
                return str_sb[:, o:o + NUNITS * HD]

            def wv(e):
                o = SW * e + TOK + NUNITS * HD
                return str_sb[:, o:o + VC]

            # persistent intermediates
            qkp_sb = [[None] * NSEG for _ in range(5)]
            qrot = [P.tile([HD, TOK], bf16, name=f"qrot{u}", tag=f"qrot{u}")
                    for u in range(NUNITS)]
            v_sb = [P.tile([128, VTOT], bf16, name=f"vsb{m}", tag=f"vsb{m}")
                    for m in range(TOK // 128)]
            # repacked ctx: 3 tiles of 128 rows per segment (last 64 zero)
            ctxp = [[P.tile([128, SEGLEN], bf16, name=f"ctxp{c}_{s}",
                            tag=f"ctxp{c}_{s}") for c in range(3)]
                    for s in range(NSEG)]
            o_sb = [P.tile([128, KCH * SEGLEN], f16, name=f"osb{s}",
                           tag=f"osb{s}") for s in range(NSEG)]

            # PSUM budget (8 banks): qk-proj, pi-swap and out-proj share a
            # 3-slot pool; v + 1/den broadcast 1; scores 2; ctx 2.
            with tc.tile_pool(name="ps_a", bufs=3, space="PSUM") as PSA, \
                 tc.tile_pool(name="ps_v", bufs=1, space="PSUM") as PSV, \
                 tc.tile_pool(name="ps_st", bufs=2, space="PSUM") as PST, \
                 tc.tile_pool(name="ps_ctx", bufs=2, space="PSUM") as PSC, \
                 tc.tile_pool(name="work", bufs=3) as W, \
                 tc.tile_pool(name="workd", bufs=6) as WD:

                ones80 = P.tile([128, HD], f32, name="ones80", tag="ones80")
                nc.vector.memset(ones80[:], 1.0)
                # ones columns (softmax-denominator) of every v tile, once
                for m_ in range(TOK // 128):
                    for j_ in range(HPC):
                        nc.vector.memset(
                            v_sb[m_][:, VW * j_ + 96:VW * j_ + 97], 1.0)
                # zero the 64 tail rows of the last ctx-repack tiles once
                for s in range(NSEG):
                    nc.vector.memset(ctxp[s][2][64:128, :], 0.0)

                est = {}    # (s, j) -> list of 4 exp'd score tiles

                def qkproj(t, s):
                    sc = slice(SEGLEN * s, SEGLEN * (s + 1))
                    qk_ps = PSA.tile([128, SEGLEN], f32, name=f"qkps{t}_{s}",
                                     tag="mm512")
                    for e in range(KCH):
                        nc.tensor.matmul(qk_ps[:],
                                         wqk(e)[:, 128 * t:128 * (t + 1)],
                                         xt(e)[:, sc],
                                         start=(e == 0), stop=(e == KCH - 1))
                    q_sb = W.tile([128, SEGLEN], bf16, name=f"qsb{t}_{s}",
                                  tag="qsb", bufs=10)
                    nc.scalar.activation(q_sb[:], qk_ps[:], AF.Identity,
                                         bias=bias_sb[:, t:t + 1])
                    qkp_sb[t][s] = q_sb

                def vchunk(m):
                    mc = slice(128 * m, 128 * (m + 1))
                    v_ps = PSV.tile([128, VC], f32, name=f"vps{m}", tag="vps")
                    for e in range(KCH):
                        nc.tensor.matmul(v_ps[:], xt(e)[:, mc], wv(e),
                                         start=(e == 0), stop=(e == KCH - 1))
                    # scatter compact 80-col head blocks into the 97-stride
                    # v layout, adding the v-bias row pattern
                    dst = v_sb[m][:, :].rearrange("p (j c) -> p j c",
                                                  c=VW)[:, :, 0:HD]
                    nc.vector.scalar_tensor_tensor(
                        dst, v_ps[:].rearrange("p (j c) -> p j c", c=HD),
                        1.0, vpat_sb[:].rearrange("p (j c) -> p j c", c=HD),
                        ALU.mult, ALU.add)

                def rotary(tr, s):
                    sc = slice(SEGLEN * s, SEGLEN * (s + 1))
                    qsw_ps = PSA.tile([128, SEGLEN], f32, name=f"qsw{tr}_{s}",
                                      tag="mm512")
                    srcs = PI_BLOCKS[tr]
                    for i, tp in enumerate(srcs):
                        nc.tensor.matmul(qsw_ps[:],
                                         pit_sb[:, NUNITS * HD * tp + 128 * tr:
                                                NUNITS * HD * tp + 128 * (tr + 1)],
                                         qkp_sb[tp][s][:],
                                         start=(i == 0),
                                         stop=(i == len(srcs) - 1))
                    t1 = W.tile([128, SEGLEN], bf16, name=f"t1_{tr}_{s}",
                                tag="t1", bufs=4)
                    nc.vector.tensor_tensor(t1[:], qkp_sb[tr][s][:],
                                            cos_sb[:, TOK * tr + sc.start:
                                                   TOK * tr + sc.stop],
                                            ALU.mult)
                    t2 = W.tile([128, SEGLEN], bf16, name=f"t2_{tr}_{s}",
                                tag="t2", bufs=4)
                    nc.vector.tensor_tensor(t2[:], qsw_ps[:],
                                            sin_sb[:, TOK * tr + sc.start:
                                                   TOK * tr + sc.stop],
                                            ALU.mult)
                    rp = W.tile([128, SEGLEN], bf16, name=f"rotp{tr}_{s}",
                                tag="rotp", bufs=6)
                    nc.vector.tensor_tensor(rp[:], t1[:], t2[:], ALU.add)
                    for (u, po, toff, ln) in UNPACK_PIECES[tr]:
                        nc.sync.dma_start(qrot[u][po:po + ln, sc],
                                          rp[toff:toff + ln, :])

                def scores(s, j):
                    sc = slice(SEGLEN * s, SEGLEN * (s + 1))
                    lst = []
                    for tkc in range(SEGLEN // 128):
                        kc = slice(SEGLEN * s + 128 * tkc,
                                   SEGLEN * s + 128 * (tkc + 1))
                        st_ps = PST.tile([128, SEGLEN], f32,
                                         name=f"st{j}_{s}_{tkc}", tag="stps")
                        nc.tensor.matmul(st_ps[:], qrot[2 * j + 1][:, kc],
                                         qrot[2 * j][:, sc],
                                         start=True, stop=True)
                        e_sb = WD.tile([128, SEGLEN], bf16,
                                       name=f"est{j}_{s}_{tkc}", tag="est",
                                       bufs=14)
                        nc.scalar.activation(e_sb[:], st_ps[:], AF.Exp)
                        lst.append(e_sb)
                    est[(s, j)] = lst

                def ctx(s, j):
                    lst = est.pop((s, j))
                    ctx_ps = PSC.tile([128, SEGLEN], f32, name=f"ctxps{j}_{s}",
                                      tag="ctxps")
                    for tkc in range(SEGLEN // 128):
                        nc.tensor.matmul(ctx_ps[0:VW, :],
                                         v_sb[4 * s + tkc][:, VW * j:VW * (j + 1)],
                                         lst[tkc][:],
                                         start=(tkc == 0), stop=(tkc == 3))
                    # den accumulated into partition 96 (ones column of the
                    # v block); 1/den broadcast to 0..79 via K=1 matmul
                    rec = WD.tile([128, SEGLEN], f32, name=f"rec{j}_{s}",
                                  tag="rec", bufs=2)
                    nc.vector.reciprocal_approx_fast(rec[:, :], ctx_ps[:, :])
                    bc_ps = PSV.tile([HD, SEGLEN], f32, name=f"bc{j}_{s}",
                                     tag="vps")
                    nc.tensor.matmul(bc_ps[:], ones80[96:97, :], rec[96:97, :],
                                     start=True, stop=True,
                                     tile_position=(96, 0))
                    ctx_sb = WD.tile([HD, SEGLEN], f32, name=f"ctxsb{j}_{s}",
                                     tag="ctxsb", bufs=2)
                    if j % 2 == 0:
                        nc.scalar.activation(ctx_sb[:], ctx_ps[0:HD, :],
                                             AF.Identity)
                    else:
                        nc.vector.tensor_copy(ctx_sb[:], ctx_ps[0:HD, :])
                    ctxn = WD.tile([HD, SEGLEN], bf16, name=f"ctxn{j}_{s}",
                                   tag="ctxn", bufs=3)
                    nc.vector.tensor_tensor(ctxn[:], ctx_sb[:], bc_ps[:],
                                            ALU.mult)
                    # repack into the 3 K=128 out-proj tiles
                    for (po, c, toff, ln) in CTXP_PIECES[j]:
                        nc.sync.dma_start(ctxp[s][c][toff:toff + ln, :],
                                          ctxn[po:po + ln, :])

                def oproj(e, s):
                    o_ps = PSA.tile([128, SEGLEN], f32, name=f"ops{e}_{s}",
                                    tag="mm512")
                    for c in range(3):
                        nc.tensor.matmul(o_ps[:],
                                         wp_sb[:, EMBED * c + 128 * e:
                                               EMBED * c + 128 * (e + 1)],
                                         ctxp[s][c][:],
                                         start=(c == 0), stop=(c == 2))
                    oc = o_sb[s][:, SEGLEN * e:SEGLEN * (e + 1)]
                    if e % 2 == 0:
                        nc.vector.tensor_copy(oc, o_ps[:])
                    else:
                        nc.scalar.activation(oc, o_ps[:], AF.Identity)

                def ostore(s, e0, e1):
                    src = o_sb[s][:, SEGLEN * e0:SEGLEN * e1].rearrange(
                        "p (e t) -> p e t", t=SEGLEN)
                    dst = out_d[128 * e0:128 * e1,
                                SEGLEN * s:SEGLEN * (s + 1)].rearrange(
                        "(e p) t -> p e t", p=128)
                    nc.sync.dma_start(dst, src)

                # ---- segment-serialized phases: all of seg-0's
                # projection + rotary + attention first (attention starts
                # after ~3 packed tiles), then seg-1's pass doubles as PE
                # filler for seg-0's out-proj and vice versa ----
                VCHS = {0: {0: [0, 1], 1: [2, 3]}, 1: {0: [4, 5], 1: [6, 7]}}

                def bpass(s):
                    for t in range(5):
                        qkproj(t, s)
                        for m in VCHS[s].get(t, []):
                            vchunk(m)
                        ready = [t - 1] if t < 4 else [3, 4]
                        for tr in ready:
                            if tr >= 0:
                                rotary(tr, s)
                        if t >= 2:
                            scores(s, t - 2)
                            ctx(s, t - 2)
                            if s == 1:
                                oproj(2 * (t - 2), 0)
                                oproj(2 * (t - 2) + 1, 0)
                    scores(s, 3)
                    ctx(s, 3)

                bpass(0)
                bpass(1)
                ostore(0, 0, 5)
                oproj(6, 0)
                oproj(7, 0)
                oproj(8, 0)
                oproj(9, 0)
                ostore(0, 5, KCH)
                for e in range(KCH):
                    oproj(e, 1)
                    if e == 4:
                        ostore(1, 0, 5)
                    if e == 8:
                        ostore(1, 5, 8)
                ostore(1, 8, KCH)

    nc.compile()
    return nc


def _prep_inputs(x, rotary_pos_emb, qkv_w, qkv_b):
    """Build per-core input shards (host-side layout/constant prep)."""
    x2 = np.asarray(x, np.float32).reshape(SEQ, EMBED)
    rope = np.asarray(rotary_pos_emb, np.float32)
    qkv_w = np.asarray(qkv_w, np.float32)
    qkv_b = np.asarray(qkv_b, np.float32)

    # packed rotary multipliers: packed row p = 80u + d -> r = d % 40
    r_idx = np.tile(np.arange(HD) % RH, NUNITS)      # [640]
    cos_full = np.cos(rope)[:, r_idx].T.astype(BF)   # [640, 2048]
    sin_full = np.sin(rope)[:, r_idx].T.astype(BF)

    # packed swap permutation (sign folded), block-diagonal per 80-row unit
    D = NUNITS * HD
    Pi = np.zeros((D, D), np.float32)
    for u in range(NUNITS):
        o = HD * u
        for i in range(RH):
            Pi[o + i, o + i + RH] = -1.0
            Pi[o + i + RH, o + i] = 1.0
    pit = np.ascontiguousarray(Pi.T).astype(BF)

    in_maps = []
    for c in range(N_CORES):
        sg, hg = divmod(c, HPC)
        toks = slice(TOK * sg, TOK * (sg + 1))
        heads = [HPC * hg + j for j in range(HPC)]

        xa = x2[toks].T                                   # [1280, 1024]

        # interleaved packing: unit 2j = q of head j, unit 2j+1 = k
        wqk = np.empty((EMBED, NUNITS * HD), np.float32)
        bias_flat = np.empty(NUNITS * HD, np.float32)
        for j, h in enumerate(heads):
            oq, ok = HD * 2 * j, HD * (2 * j + 1)
            wqk[:, oq:oq + HD] = qkv_w[HD * h:HD * (h + 1), :].T * SCALE
            bias_flat[oq:oq + HD] = qkv_b[HD * h:HD * (h + 1)] * SCALE
            ko = EMBED + HD * h
            wqk[:, ok:ok + HD] = qkv_w[ko:ko + HD, :].T
            bias_flat[ok:ok + HD] = qkv_b[ko:ko + HD]
        bias = np.ascontiguousarray(bias_flat.reshape(5, 128).T)

        # compact v weights (80 cols per head); the eviction scatters them
        # into 97-wide blocks whose col 96 is a memset ones column that makes
        # the ctx matmul accumulate the softmax denominator at partition 96
        wv = np.zeros((EMBED, VC), np.float32)
        vpat_row = np.zeros(VC, np.float32)
        for j, h in enumerate(heads):
            vo = 2 * EMBED + HD * h
            wv[:, HD * j:HD * (j + 1)] = qkv_w[vo:vo + HD, :].T
            vpat_row[HD * j:HD * (j + 1)] = qkv_b[vo:vo + HD]
        vpat = np.ascontiguousarray(np.broadcast_to(vpat_row, (128, VC)))

        stream = np.concatenate([xa, wqk, wv], axis=1)    # [1280, SW]

        # wp packed for K=128 repacked ctx: rows = stacked head-dims
        # (320 real + 64 zero), 3 chunks of 128 side by side
        wp_cat = np.zeros((384, EMBED), np.float32)
        for j, h in enumerate(heads):
            wp_cat[HD * j:HD * (j + 1), :] = _PROJ_W[:, HD * h:HD * (h + 1)].T
        wp = np.zeros((128, 3 * EMBED), np.float32)
        for c_ in range(3):
            wp[:, EMBED * c_:EMBED * (c_ + 1)] = wp_cat[128 * c_:128 * (c_ + 1)]

        in_maps.append({
            "stream": np.ascontiguousarray(stream).astype(BF),
            "vpat": vpat.astype(BF),
            "wp": np.ascontiguousarray(wp).astype(BF),
            "biasqk": bias,
            "cosm": np.ascontiguousarray(cos_full[:, toks]),
            "sinm": np.ascontiguousarray(sin_full[:, toks]),
            "pit": pit,
        })
    return in_maps


_PROJ_W = None


def run_on_device(inputs, trace=False, trace_cores=None):
    """Shard, run on 8 NeuronCores, gather. Returns (output, BassKernelResults)."""
    global _PROJ_W
    from concourse import bass_utils

    x = np.asarray(inputs["x"], np.float32)
    cu = np.asarray(inputs["cu_seqlens"]).tolist()
    assert cu == [0, 512, 1024, 1536, 2048], (
        f"kernel compiled for 4x512 segments, got cu_seqlens={cu}")
    assert x.shape == (SEQ, 1, EMBED)

    _PROJ_W = np.asarray(inputs["proj_w"], np.float32)
    in_maps = _prep_inputs(x, inputs["rotary_pos_emb"],
                           inputs["qkv_w"], inputs["qkv_b"])

    if "nc" not in _CACHE:
        _CACHE["nc"] = _build_program()
    nc = _CACHE["nc"]

    kw = {}
    if trace:
        kw = dict(trace=True, trace_cores=trace_cores or [0])
    res = bass_utils.run_bass_kernel_spmd(nc, in_maps,
                                          core_ids=list(range(N_CORES)), **kw)

    proj_b = np.asarray(inputs["proj_b"], np.float32)
    out = np.empty((SEQ, EMBED), np.float32)
    for sg in range(2):
        acc = res.results[HPC * sg + 0]["outT"].astype(np.float32)
        for hg in range(1, HPC):
            acc = acc + res.results[HPC * sg + hg]["outT"].astype(np.float32)
        out[TOK * sg:TOK * (sg + 1)] = acc.T
    out += proj_b
    return out.reshape(SEQ, 1, EMBED), res


def kernel(**inputs):
    out, _ = run_on_device(inputs, trace=False)
    return out
